# revision 29
# baseline (speedup 1.0000x reference)
"""Trainium2 Bass kernel for nn_Attention_40716289966507.

Reference computation (B=4, C=256, H=W=48, heads=8, d=32, N=H*W=2304):
    qkv = w_qkv @ x            # 1x1 conv -> q,k,v each [B, 256, N]
    attn = softmax(q^T k / sqrt(d))   per (batch, head): [N, N]
    out  = v @ attn^T          # [B, 256, N]
    y    = w_proj @ out + b    # [B, 256, N]

Sharding (8 cores): core i handles batch b = i//2 and query-token half
t = i%2 (1152 of the 2304 tokens). The host feeds each core its batch
image ROLLED so the core's query tokens sit at columns 0:1152 (keys are
permutation-invariant under softmax+AV, so rolling is harmless); the
core outputs the full 256-channel result for its 1152 tokens and the
host concatenates.

v2 structural change vs v1: the softmax denominators are folded into
the AV matmuls. AV weights per head are lhsT = [v_h | ones32] (M=64,
col-tiled at PSUM partition 0/64), so each probability tile streams
through the PE ONCE and yields both the 32 output rows and the
denominator replicated over 32 rows. This removes the entire
ones-matmul (sm) pack - measured ~150ns/col-matmul on HW, i.e. ~48us
of TensorE queue time. Normalization (offset-aligned for walrus: custom
DVE ops and operand partition offsets must match / be zero): one
full-tile f32 reciprocal per pass, a DMA that shifts the den-reciprocal
rows down 32 partitions, and one fused multiply into a 4-quarter fp16
av_sb whose den/junk rows pair with zero rows in the host-padded proj
weights. The multiply is DEFERRED into the next pass (after its kc==0
exps) so the DMA latency never stalls the DVE queue. Projection runs
in fp16 over the 4 quarters.

exp: custom 8-stage DVE op EXP32_ANT computes exp in ONE VectorE
instruction, exp(z) ~= ((w + A)*w + B)^32 with w = S*z folded into the
host-side q weights; ScalarE computes exp(w/S) via its free affine.
The two engines split the 180 exp tiles by ACT_FRAC.

PSUM budget 8 banks: st 2x2 + av 2 + qkv 2.
emit(tc, reps=N) wraps the body in a For_i hardware loop for the
timing NEFF used by test.py's slope estimator (kernel() uses reps=1,
no loop).
"""

import numpy as np

import concourse.bacc as bacc
import concourse.mybir as mybir
import concourse.tile as tile
from concourse import dve_ops
from concourse.dve_spec import Spec, Src0, C0, C1, sq, lower
from concourse.dve_uop import DveOpSpec

F32 = mybir.dt.float32
F32R = mybir.dt.float32r
FP16 = mybir.dt.float16

P = 128
C = 256          # channels
N = 2304         # tokens per image
NQ = 1152        # query tokens per core
D = 32           # head dim
KC = N // P      # 18 key chunks
QT = 384         # query tile (3 per group); >=256 keeps f32r full-rate
NT = NQ // QT    # 3
SCALE = D ** -0.5

# exp(z) ~= ((S*z + A)*(S*z) + B)^32, minimax-fitted on z in [-6.2, 6.2]
S_EXP = 0.02203952907337898
A_EXP = 1.4221366019241177
B_EXP = 1.0000287417426525

# fraction of exp tiles on ScalarE (rest on VectorE custom op)
ACT_FRAC = 0.525

# diagnostic-only: skip the norm reciprocal-shift DMAs (timing NEFFs only;
# output is numerically wrong when False)
_NORM_DMA = True


def _register_exp32():
    name = "EXP32_ANT"
    for op in dve_ops.OPS:
        if op.name == name:
            return op

    def _ref(in0, in1, c0, c1, c2):
        q = ((in0.astype(np.float32) + np.float32(c0)) * in0
             + np.float32(c1)).astype(np.float32)
        for _ in range(5):
            q = (q * q).astype(np.float32)
        return q

    spec = Spec(body=sq(sq(sq(sq(sq((Src0 + C0) * Src0 + C1))))),
                reference=_ref)
    row = max(dve_ops._SUB_OPCODE_FOR_NAME.values()) + 1
    dve_ops._SUB_OPCODE_FOR_NAME[name] = row
    shas = {}
    for ver in ("v3", "v4"):
        shas[ver] = DveOpSpec(name=name, opcode=row,
                              uops=lower(spec, ver=ver),
                              rd1_en=False).sha(ver)
    op = dve_ops.DveOp(name, spec, subdim=False, uops_sha=shas)
    dve_ops.OPS.append(op)
    dve_ops.CUSTOM_DVE_SPECS[name] = spec
    return op


EXP32 = _register_exp32()


def emit(tc, reps=1):
    from contextlib import ExitStack
    ctx = ExitStack()
    nc = tc.nc
    xf_d = nc.dram_tensor("xf", [C, N], F32R, kind="ExternalInput").ap()
    wqkvT_d = nc.dram_tensor("wqkvT", [C, 3 * C], F32R, kind="ExternalInput").ap()
    # proj weights in 4 zero-padded quarters matching av_sb's PSUM-native
    # row layout (rows 32:64 and 96:128 of each quarter are zero)
    wprojT_d = nc.dram_tensor("wprojT", [4 * P, C], FP16, kind="ExternalInput").ap()
    bprojT_d = nc.dram_tensor("bprojT", [P, 2], F32, kind="ExternalInput").ap()
    y_d = nc.dram_tensor("y", [C, NQ], F32, kind="ExternalOutput").ap()

    singles = ctx.enter_context(tc.tile_pool(name="singles", bufs=1))
    acts = ctx.enter_context(tc.tile_pool(name="acts", bufs=1))
    qkv_ps = ctx.enter_context(tc.tile_pool(name="qkv_ps", bufs=2, space="PSUM"))
    st_ps = ctx.enter_context(tc.tile_pool(name="st_ps", bufs=2, space="PSUM"))
    av_ps = ctx.enter_context(tc.tile_pool(name="av_ps", bufs=1, space="PSUM"))
    pt_pool = ctx.enter_context(tc.tile_pool(name="pt", bufs=4))
    small = ctx.enter_context(tc.tile_pool(name="small", bufs=2))

    # preload the exp table while DMAs/qkv run
    warm = singles.tile([P, 8], F32)
    nc.vector.memset(warm[:], 0.0)
    warm2 = singles.tile([P, 8], F32)
    nc.scalar.activation(warm2[:], warm[:], mybir.ActivationFunctionType.Exp)

    bias_sb = singles.tile([P, 2], F32)
    nc.sync.dma_start(bias_sb[:], bprojT_d)

    # weights: per-ki-chunk DMAs for early starts
    wq_sb = singles.tile([P, 2, 3 * C], F32R)
    wqkvT_r = wqkvT_d.rearrange("(ki p) o -> p ki o", p=P)

    def emit_w_dma(sec):
        for ki in range(2):
            sl = slice(sec * C, (sec + 1) * C)
            nc.sync.dma_start(wq_sb[:, ki, sl], wqkvT_r[:, ki, sl])

    wp_sb = singles.tile([P, 4, C], FP16)

    # x: full image (rolled so this core's queries are cols 0:NQ)
    xf_sb = singles.tile([P, 2, N], F32R)
    xf_r = xf_d.rearrange("(ki p) n -> p ki n", p=P)

    def emit_x_dma():
        # nt-major so qkv tile nt is ready after its own two sub-DMAs
        for nt in range(N // QT):
            for ki in range(2):
                sl = slice(nt * QT, (nt + 1) * QT)
                nc.sync.dma_start(xf_sb[:, ki, sl], xf_r[:, ki, sl])

    emit_w_dma(0)                 # q weights first: the first qkv matmul
    if reps == 1:                 # needs only these + x tile 0
        emit_x_dma()              # single-shot: x right behind q weights
    for sec in (1, 2):
        emit_w_dma(sec)
    nc.sync.dma_start(wp_sb[:], wprojT_d.rearrange("(ki p) o -> p ki o", p=P))

    # per-group activations
    q_g = [acts.tile([P, NQ], FP16, name=f"q{g}") for g in range(2)]
    k_g = [acts.tile([P, N], FP16, name=f"k{g}") for g in range(2)]
    # [v | ones] augmented AV weights: [keys, chunk, 8 heads, 64]
    vT_all = acts.tile([P, KC, 8, 64], FP16, name="vt")
    nc.vector.memset(vT_all[:, :, :, D:2 * D], 1.0)
    # av_sb quarters (2g + j): rows 0:32 = head(4g+2j) nums, 32:64 = zeros,
    # 64:96 = head(4g+2j+1) nums, 96:128 = never written (memset zeros);
    # the zero rows pair with zero rows in the host-padded proj weights.
    av_sb = acts.tile([P, 4, NQ], FP16)
    nc.vector.memset(av_sb[:], 0.0)
    # shifted reciprocal tile: rows {0:32, 64:96} are DMA-refreshed per
    # pass, rows {32:64, 96:128} stay zero from this memset
    rcs_sb = acts.tile([P, 2, 512], F32)
    nc.vector.memset(rcs_sb[:], 0.0)
    y_sb = acts.tile([P, 2, NQ], F32)

    mm = nc.tensor.matmul

    # exp engine schedule: deterministic Bresenham on ACT_FRAC
    exp_acc = [0.0]

    def exp_engine():
        exp_acc[0] += ACT_FRAC
        if exp_acc[0] >= 1.0:
            exp_acc[0] -= 1.0
            return "act"
        return "dve"

    def qkv_mm(dst_tile, w_col0, nt, evac):
        sl = slice(nt * QT, (nt + 1) * QT)
        pst = qkv_ps.tile([P, 512], F32, tag="qkv", name="qkvp")
        ps = pst[:, :QT]
        for ki in range(2):
            mm(ps, wq_sb[:, ki, w_col0:w_col0 + P], xf_sb[:, ki, sl],
               start=(ki == 0), stop=(ki == 1))
        if evac == "act":
            nc.scalar.copy(dst_tile[:, sl], ps)
        else:
            nc.vector.tensor_copy(dst_tile[:, sl], ps)

    def emit_qkv_group(g):
        # q rows for group g = channels 128g..128g+127; k = 256+128g..
        for nt in range(NT):
            qkv_mm(q_g[g], g * P, nt, "dve")
        for nt in range(N // QT):
            qkv_mm(k_g[g], C + g * P, nt, "act")

    def emit_vt(mo2):
        # two key-chunks (2*mo2, 2*mo2+1) share one PSUM tile so the
        # evacuation is a single FD-512 copy instead of two FD-256 ones
        pst = qkv_ps.tile([P, 512], F32, tag="qkv", name="qkvp")
        ps = pst[:, :]
        for half in range(2):
            mo = 2 * mo2 + half
            for ki in range(2):
                mm(ps[:, half * C:(half + 1) * C],
                   xf_sb[:, ki, mo * P:(mo + 1) * P],
                   wq_sb[:, ki, 2 * C:3 * C],
                   start=(ki == 0), stop=(ki == 1))
        # strided copy into the v columns of [v | ones] (free sizes match;
        # dst walks (chunk, head, d) exactly as src walks the two chunks'
        # channels 256*half + 32h + d)
        nc.vector.tensor_copy(vT_all[:, 2 * mo2:2 * mo2 + 2, :, 0:D], ps)

    def emit_exp(pt, st, pair, width):
        if exp_engine() == "act":
            nc.scalar.activation(pt[:, 2 * pair:2 * pair + 2, :width],
                                 st[:, :, :width],
                                 mybir.ActivationFunctionType.Exp,
                                 scale=1.0 / S_EXP)
        else:
            nc.vector._custom_dve(EXP32,
                                  out=pt[:, 2 * pair:2 * pair + 2, :width],
                                  in0=st[:, :, :width],
                                  s0=A_EXP, s1=B_EXP)

    # Deferred normalization: the reciprocal + shift-DMAs are emitted at a
    # pass's end, but the multiply is deferred (pending_norm) into the NEXT
    # pass, right after its kc==0 exp instructions and before its AV
    # matmuls, so the DVE queue never stalls on DMA latency. (Deadlock-free:
    # the mul only precedes, in DVE program order, instructions whose PE
    # producers are themselves ahead of anything waiting on the mul.)
    pending_norm = []

    def flush_norm():
        while pending_norm:
            pending_norm.pop(0)()

    def emit_norm(av, g, q0, qtw, tail=False):
        # av bank j rows: [num(2j) | den(2j) | num(2j+1) | den(2j+1)], each
        # den replicated over 32 rows by the ones half of [v | ones].
        # Every engine op keeps identical partition offsets on all operands
        # (walrus silently miscompiles offset-mismatched DVE ops); the only
        # partition move is a DMA of the reciprocals down 32 rows. Custom
        # DVE ops also miscompile at non-zero partition offsets, so the
        # reciprocal covers the WHOLE tile at offset 0; rows holding 1/num
        # are garbage (possibly inf) and are never read.
        w = qtw if not tail else 2 * qtw
        rc = small.tile([P, 2, 512], F32, tag="rc")
        nc.vector.reciprocal_approx_fast(rc[:, :, :w], av[:, :, :w])
        if _NORM_DMA:
            nc.sync.dma_start(rcs_sb[0:D, :, :w], rc[D:2 * D, :, :w])
            nc.sync.dma_start(rcs_sb[64:96, :, :w], rc[96:P, :, :w])

        def mul():
            if not tail:
                nc.vector.tensor_mul(av_sb[0:96, 2 * g:2 * g + 2,
                                           q0:q0 + qtw],
                                     av[0:96, :, :qtw],
                                     rcs_sb[0:96, :, :qtw])
            else:
                for gg in range(2):
                    nc.vector.tensor_mul(
                        av_sb[0:96, 2 * gg:2 * gg + 2, q0:q0 + qtw],
                        av[0:96, :, gg * qtw:(gg + 1) * qtw],
                        rcs_sb[0:96, :, gg * qtw:(gg + 1) * qtw])

        pending_norm.append(mul)

    def emit_attention(g, q0, qtw, lazy_kv=False):
        # lazy_kv (first pass only): emit group-0 k-evacs and v-chunk
        # builds just before the key-chunk that needs them, so the exp
        # engines start ~13us earlier instead of idling behind the whole
        # qkv phase in the in-order PE queue.
        state = {"k": 0, "vt": 0}

        def ensure_kv(kc):
            if not lazy_kv:
                return
            while state["k"] * QT < (kc + 1) * P and state["k"] < N // QT:
                qkv_mm(k_g[0], C, state["k"], "act")
                state["k"] += 1
            while state["vt"] <= kc // 2 and state["vt"] < KC // 2:
                emit_vt(state["vt"])
                state["vt"] += 1

        av = None
        for kc in range(KC):
            ensure_kv(kc)
            pt = pt_pool.tile([P, 4, 512], FP16)
            for pair in range(2):
                st = st_ps.tile([P, 2, 512], F32, tag="st")
                for hh in range(2):
                    h = 2 * pair + hh
                    mm(st[:, hh, :qtw],
                       k_g[g][D * h:D * (h + 1), kc * P:(kc + 1) * P],
                       q_g[g][D * h:D * (h + 1), q0:q0 + qtw],
                       start=True, stop=True,
                       tile_position=(D * h, 0))
                emit_exp(pt, st, pair, qtw)
            if kc == 0:
                flush_norm()
                av = av_ps.tile([P, 2, 512], F32, name="av")
            for h in range(4):
                mm(av[64 * (h % 2):64 * (h % 2) + 64, h // 2, :qtw],
                   vT_all[:, kc, 4 * g + h, :],
                   pt[:, h, :qtw],
                   start=(kc == 0), stop=(kc == KC - 1),
                   tile_position=(0, 64 * (h % 2)), skip_group_check=True)
        emit_norm(av, g, q0, qtw)

    def emit_tail(mid_cb=None):
        # queries 1024:1152 for BOTH groups in one pass: head slot h holds
        # g0 at cols 0:128, g1 at cols 128:256
        q0, qtw = 1024, 128
        av = None
        for kc in range(KC):
            pt = pt_pool.tile([P, 4, 512], FP16)
            for pair in range(2):
                st = st_ps.tile([P, 2, 512], F32, tag="st")
                for g in range(2):
                    for hh in range(2):
                        h = 2 * pair + hh
                        mm(st[:, hh, g * qtw:(g + 1) * qtw],
                           k_g[g][D * h:D * (h + 1), kc * P:(kc + 1) * P],
                           q_g[g][D * h:D * (h + 1), q0:q0 + qtw],
                           start=(g == 0), stop=(g == 1),
                           tile_position=(D * h, 0), skip_group_check=True)
                emit_exp(pt, st, pair, 2 * qtw)
            if kc == 0:
                flush_norm()
                av = av_ps.tile([P, 2, 512], F32, name="av")
            if kc == 1 and mid_cb is not None:
                mid_cb()
            for g in range(2):
                for h in range(4):
                    mm(av[64 * (h % 2):64 * (h % 2) + 64, h // 2,
                          g * qtw:(g + 1) * qtw],
                       vT_all[:, kc, 4 * g + h, :],
                       pt[:, h, g * qtw:(g + 1) * qtw],
                       start=(kc == 0 and g == 0),
                       stop=(kc == KC - 1 and g == 1),
                       tile_position=(0, 64 * (h % 2)), skip_group_check=True)
        emit_norm(av, None, q0, qtw, tail=True)

    y_r = y_d.rearrange("(co p) n -> p co n", p=P)

    def emit_proj(co, nt):
        flush_norm()
        sl = slice(nt * QT, (nt + 1) * QT)
        pst = qkv_ps.tile([P, 512], F32, tag="qkv", name="qkvp")
        ps = pst[:, :QT]
        for q4 in range(4):
            mm(ps, wp_sb[:, q4, co * P:(co + 1) * P],
               av_sb[:, q4, sl],
               start=(q4 == 0), stop=(q4 == 3))
        nc.scalar.add(y_sb[:, co, sl], ps, bias_sb[:, co:co + 1])
        nc.sync.dma_start(y_r[:, co, sl], y_sb[:, co, sl])

    def body():
        # projs are placed so every norm-mul they need was already flushed
        # inside a following attention pass (the DMA latency hides behind
        # that pass's kc==0 exps); emit_proj's own flush covers the rest.
        if reps > 1:
            emit_x_dma()
        for nt in range(NT):
            qkv_mm(q_g[0], 0, nt, "dve")
        emit_attention(0, 0, 512, lazy_kv=True)
        emit_qkv_group(1)
        emit_attention(1, 0, 512)       # flushes norm(g0, 0:512)
        emit_attention(0, 512, 512)     # flushes norm(g1, 0:512)
        emit_proj(0, 0)
        emit_proj(1, 0)
        emit_attention(1, 512, 512)     # flushes norm(g0, 512:1024)
        emit_tail(mid_cb=lambda: (emit_proj(0, 1), emit_proj(1, 1)))
        emit_proj(0, 2)                 # flushes norm(tail)
        emit_proj(1, 2)

    if reps == 1:
        body()
    else:
        with tc.For_i(0, reps):
            body()
    ctx.close()


_NC_CACHE = {}


def build_nc(reps=1):
    if reps not in _NC_CACHE:
        nc = bacc.Bacc("TRN2", target_bir_lowering=False, debug=False,
                       num_devices=8)
        with tile.TileContext(nc) as tc:
            emit(tc, reps=reps)
        nc.compile()
        _NC_CACHE[reps] = nc
    return _NC_CACHE[reps]


def build_timing_nc(reps=4):
    return build_nc(reps)


def make_in_maps(x, w_qkv, w_proj, b_proj):
    x = np.ascontiguousarray(np.asarray(x, np.float32)).reshape(4, C, N)
    wqkvT = np.asarray(w_qkv, np.float32).T.copy()
    wqkvT[:, :C] *= np.float32(SCALE * S_EXP)   # fold softmax scale + S into q
    wpT = np.asarray(w_proj, np.float32).T  # [in_ch, out_ch]
    # 4 quarters (g, j): rows 0:32 = in-ch 128g+64j+(0:32), 64:96 =
    # in-ch 128g+64j+32+(0:32), rows 32:64 and 96:128 zero (they pair
    # with the den/junk rows of av_sb)
    wprojT = np.zeros((4 * P, C), np.float16)
    for g in range(2):
        for j in range(2):
            q4 = 2 * g + j
            base = 128 * g + 64 * j
            wprojT[q4 * P:q4 * P + D] = wpT[base:base + D].astype(np.float16)
            wprojT[q4 * P + 64:q4 * P + 96] = \
                wpT[base + D:base + 2 * D].astype(np.float16)
    wprojT = np.ascontiguousarray(wprojT)
    bprojT = np.ascontiguousarray(np.asarray(b_proj, np.float32).reshape(2, P).T)
    in_maps = []
    for core in range(8):
        b, t = divmod(core, 2)
        in_maps.append({
            "xf": np.ascontiguousarray(np.roll(x[b], -t * NQ, axis=1)),
            "wqkvT": wqkvT,
            "wprojT": wprojT,
            "bprojT": bprojT,
        })
    return in_maps


def assemble_output(results):
    y = np.empty((4, C, N), np.float32)
    for core in range(8):
        b, t = divmod(core, 2)
        y[b][:, t * NQ:(t + 1) * NQ] = results[core]["y"]
    return y.reshape(4, C, 48, 48)


def kernel(x, w_qkv, w_proj, b_proj):
    from concourse.bass_utils import run_bass_kernel_spmd
    nc = build_nc()
    in_maps = make_in_maps(x, w_qkv, w_proj, b_proj)
    res = run_bass_kernel_spmd(nc, in_maps, core_ids=list(range(8)))
    return assemble_output(res.results)
